# revision 32
# baseline (speedup 1.0000x reference)
"""CNN attention (nn_CNNAttention_77979426226593) Trainium2 Bass kernel.

Data-parallel over batch: B=16 images -> 8 NeuronCores, 2 images per core.
Each core holds the full (small) conv1x1 weights and computes its local
N x N attention (N = H*W = 4096) independently.

Per image (C=256, N=4096, CQK=32):
  q = wq @ x + bq            [32, N]
  k = wk @ x + bk            [32, N]
  vt = x^T @ [wv^T | 0] + [bv | 1]   [N, 257]  (V transposed, plus a ones
                                     column that rides along as channel 256)
  T[n, m] = k_n . q_m        (scores, N x N, computed n-partitioned)
  E = exp(T)                 (no max-subtraction: logits are small by
                              construction, exp fits fp32/bf16 easily)
  UT[m, c] = sum_n E[n, m] * vt[n, c]   for c in 0..256
       -> UT[:, 0:256] = U^T (attention numerator, m-partitioned)
       -> UT[:, 256]   = d   (softmax denominator) for FREE: the ones
          column of vt adds 1 moving column (~0.4%) instead of a separate
          ones-matmul (which would cost a full third of the U stage,
          since PE matmul time = moving free size, independent of K).
  out[c, m] = (gamma/d[m]) * UT[m, c]^T + x[c, m]

The U matmuls put E (n-partitioned [128,128] chunks) in the STATIONARY
slot and vt in the MOVING slot; PSUM accumulates UT[m, 0:257] over all
32 n-chunks.  The [m, c] -> [c, m] flip at the end is 2 cheap PE
transposes per 128-wide m-chunk (128 cycles each, bf16).

Scores run in bf16 (4-way row-tiled K=32 matmuls -> concurrent PE
quadrants, HW-measured ~190us/exec saving vs serial); U runs in fp8
DoubleRow (E fp8e5 stationary pairs, vt fp8e4 moving, K=256/instr);
accumulation is fp32 in PSUM; softmax normalization is fp32.  The
residual term x is added from a separate fp32 copy, so when gamma == 0
the output equals the input bit-exactly.

Schedule: one "unit" = one (image, 512-wide m-tile).  Unit u runs its
own scores+exp in chunk PAIRS through a double-buffered 2-bank tp tile
(so ScalarE streams exp back-to-back - ScalarE is the pacing engine at
~276us busy/core) while the U matmuls + epilogue of unit u-1 fill the
PE gaps.  PSUM: 4 banks tp (pairs x2) + 4 banks rotating between the 4
UT accumulators and the output transposes.  Output stores issue from
the gpsimd queue so the sync queue only carries input loads (HW: -32us).
"""

import numpy as np

B, C, H, W = 16, 256, 64, 64
N = H * W          # 4096
CQK = 32
NCORES = 8
BPC = B // NCORES  # batches per core

MT = 512           # m tile (attention output columns per score quad)
NMT = N // MT      # 8
NCH = N // 128     # 32 n-chunks (contraction for U)
NQ = NCH // 4      # 8 quads per m-tile
CV = C + 1         # v channels + ones column
NU = BPC * NMT     # pipeline units

# fp8 path: E in fp8e5 (e5m2, exp shifted by E_BIAS so values stay in
# range; the shift cancels exactly in U/d), vt in fp8e4 (e4m3, |v|<~5),
# U matmuls in DoubleRow perf mode (2 fp8 weights per PE cell -> K=256
# per instruction, ~1.5x at FD>=256).  Softmax shift-invariance makes
# the E_BIAS free; accumulation stays fp32 in PSUM.
USE_FP8 = True
# exp(T + E_BIAS): max logit over this input distribution measured 26.4;
# e5m2 max finite is 57344 (ln = 10.96), so -16 keeps exp below ~e^10.4
# with margin, while the weakest softmax columns (col-max ~10) stay in
# e5m2's normal range.  The shift cancels exactly in U/d.
E_BIAS = -16.0
CVPAD = 272        # vt row stride in bytes for DoubleRow (step % 16 == 0)

# ScalarE(exp) is the pacing engine, so a subset of score pairs skips the
# activation entirely: DVE computes the e5m2 BIT PATTERN of exp(T-16)
# directly as uint8 = clamp(T*4*log2(e) + SBIAS, 0, .) — a Schraudolph
# exp whose max rel error (11.6%) equals the e5m2 rounding floor anyway.
# The 4*log2(e) score scale is pre-folded into the q projection (free),
# so the DVE pair costs ONE tensor_scalar(add SBIAS, max 0.0); ScalarE
# undoes the scale with its free activation-scale operand.  The clamp
# keeps the value in [0,120], making HW/sim int-convert edge semantics
# (wrap vs saturate) irrelevant.
SCL = 4 * 1.4426950408889634   # scores arrive as T * SCL
SBIAS = -32.04                 # calibrated for truncating convert
# HW A/B probes showed the DVE offload is a net LOSS (~+13us vs all-Act:
# the DVE read of tp inserts WAR stalls into the score/exp chain and the
# DVE pair costs ~1.6us effective vs ScalarE's 1.1us).  Keep the
# machinery (probes use it) but run everything on ScalarE.
DVE_PAIRS = frozenset()


def _build_nc(repeat=1, dve_pairs=None, u_cols=None, serial_scores=False):
    import contextlib
    import concourse.bacc as bacc
    import concourse.mybir as mybir
    import concourse.tile as tile
    import concourse.bass as bass

    if dve_pairs is None:
        dve_pairs = DVE_PAIRS
    ucv = CV if u_cols is None else u_cols

    f32 = mybir.dt.float32
    bf16 = mybir.dt.bfloat16
    fp8e5 = mybir.dt.float8e5
    fp8e4 = mybir.dt.float8e4
    AF = mybir.ActivationFunctionType
    OP = mybir.AluOpType
    e_dt = fp8e5 if USE_FP8 else bf16
    vt_dt = fp8e4 if USE_FP8 else bf16

    nc = bacc.Bacc("TRN2", target_bir_lowering=False, debug=False,
                   num_devices=NCORES)

    xb_d = nc.dram_tensor("xb", [BPC, C, N], bf16, kind="ExternalInput")
    xf_d = nc.dram_tensor("xf", [BPC, C, N], f32, kind="ExternalInput")
    wqT_d = nc.dram_tensor("wqT", [C, CQK], bf16, kind="ExternalInput")
    wkT_d = nc.dram_tensor("wkT", [C, CQK], bf16, kind="ExternalInput")
    wvT_d = nc.dram_tensor("wvT", [C, CV], bf16, kind="ExternalInput")
    bq_d = nc.dram_tensor("bq", [CQK], f32, kind="ExternalInput")
    bk_d = nc.dram_tensor("bk", [CQK], f32, kind="ExternalInput")
    bv_d = nc.dram_tensor("bv", [CV], f32, kind="ExternalInput")
    gamma_d = nc.dram_tensor("gamma", [1], f32, kind="ExternalInput")
    eye_d = nc.dram_tensor("eye", [128, 128], bf16, kind="ExternalInput")
    out_d = nc.dram_tensor("out", [BPC, C, N], f32, kind="ExternalOutput")

    def bcast_ap(handle, parts, free):
        # DRAM source AP replicated across `parts` partitions (step 0)
        return bass.AP(tensor=handle, offset=0, ap=[[0, parts], [1, free]])

    with tile.TileContext(nc) as tc:
        ctx = contextlib.ExitStack()
        with ctx:
            singles = ctx.enter_context(tc.tile_pool(name="singles", bufs=1))
            xpool = ctx.enter_context(tc.tile_pool(name="xpool", bufs=2))
            qkpool = ctx.enter_context(tc.tile_pool(name="qkpool", bufs=2))
            vtpool = ctx.enter_context(tc.tile_pool(name="vtpool", bufs=2))
            epool = ctx.enter_context(tc.tile_pool(name="epool", bufs=22))
            opool = ctx.enter_context(tc.tile_pool(name="opool", bufs=4))
            xrpool = ctx.enter_context(tc.tile_pool(name="xrpool", bufs=3))
            rpool = ctx.enter_context(tc.tile_pool(name="rpool", bufs=3))

            # --- constants / weights (once) ---
            wqT = singles.tile([C // 2, 2, CQK], bf16, tag="wqT")
            nc.gpsimd.dma_start(out=wqT, in_=wqT_d.ap().rearrange(
                "(t p) o -> p t o", p=128))
            wkT = singles.tile([C // 2, 2, CQK], bf16, tag="wkT")
            nc.gpsimd.dma_start(out=wkT, in_=wkT_d.ap().rearrange(
                "(t p) o -> p t o", p=128))
            wvT = singles.tile([C // 2, 2, CV], bf16, tag="wvT")
            nc.gpsimd.dma_start(out=wvT, in_=wvT_d.ap().rearrange(
                "(t p) o -> p t o", p=128))
            bq_sb = singles.tile([128, 1], f32, tag="bq")
            nc.gpsimd.dma_start(out=bq_sb, in_=bass.AP(
                tensor=bq_d, offset=0, ap=[[0, 4], [1, CQK]]))
            bk_sb = singles.tile([128, 1], f32, tag="bk")
            nc.gpsimd.dma_start(out=bk_sb, in_=bass.AP(
                tensor=bk_d, offset=0, ap=[[0, 4], [1, CQK]]))
            bv_row = singles.tile([128, CV], f32, tag="bvrow")
            nc.gpsimd.dma_start(out=bv_row, in_=bcast_ap(bv_d, 128, CV))
            gamma_b = singles.tile([128, 1], f32, tag="gamma")
            nc.gpsimd.dma_start(out=gamma_b, in_=bcast_ap(gamma_d, 128, 1))
            eye_sb = singles.tile([128, 128], bf16, tag="eye")
            nc.gpsimd.dma_start(out=eye_sb, in_=eye_d.ap())
            ebias = None
            if USE_FP8:
                ebias = singles.tile([128, 1], f32, tag="ebias")
                nc.vector.memset(ebias, E_BIAS)

            def body():
                # --- load x + projections for both images (prologue) ---
                xt = {}
                for b in range(BPC):
                    xt[b] = [xpool.tile([128, N], bf16, tag=f"x{h}",
                                        name=f"xt{h}_{b}") for h in range(2)]
                    for h in range(2):
                        nc.sync.dma_start(
                            out=xt[b][h],
                            in_=xb_d[b, 128 * h:128 * (h + 1), :])

                q_sb, k_sb, vt_sb = {}, {}, {}
                with tc.tile_pool(name="ppsum", bufs=2, space="PSUM") as pp, \
                     tc.tile_pool(name="vpsum", bufs=2, space="PSUM") as vp_:
                    for b in range(BPC):
                        q_sb[b] = qkpool.tile([128, N], bf16, tag="q",
                                              name=f"q_{b}")
                        k_sb[b] = qkpool.tile([128, N], bf16, tag="k",
                                              name=f"k_{b}")
                        vt_sb[b] = vtpool.tile(
                            [128, NCH, CV], vt_dt, tag="vt", name=f"vt_{b}",
                            padded_shape=[128, NCH, CVPAD] if USE_FP8
                            else None)
                        for nt in range(NMT):
                            ns = slice(nt * MT, (nt + 1) * MT)
                            qp = pp.tile([128, MT], f32, tag="qp")
                            for j in range(4):
                                for h in range(2):
                                    nc.tensor.matmul(
                                        qp[32 * j:32 * (j + 1), :],
                                        wqT[:, h, :], xt[b][h][:, ns],
                                        start=(h == 0), stop=(h == 1),
                                        tile_position=(0, 32 * j))
                            nc.vector.tensor_scalar(out=q_sb[b][:, ns], in0=qp,
                                                    scalar1=bq_sb,
                                                    scalar2=SCL if USE_FP8
                                                    else None,
                                                    op0=OP.add,
                                                    op1=OP.mult if USE_FP8
                                                    else ...)
                            kp = pp.tile([128, MT], f32, tag="kp")
                            for j in range(4):
                                for h in range(2):
                                    nc.tensor.matmul(
                                        kp[32 * j:32 * (j + 1), :],
                                        wkT[:, h, :], xt[b][h][:, ns],
                                        start=(h == 0), stop=(h == 1),
                                        tile_position=(0, 32 * j))
                            nc.vector.tensor_scalar(out=k_sb[b][:, ns], in0=kp,
                                                    scalar1=bk_sb, scalar2=None,
                                                    op0=OP.add)
                        for ni in range(NCH):
                            cs = slice(ni * 128, (ni + 1) * 128)
                            vp = vp_.tile([128, CV], f32, tag="vp")
                            for h in range(2):
                                nc.tensor.matmul(vp, xt[b][h][:, cs],
                                                 wvT[:, h, :],
                                                 start=(h == 0), stop=(h == 1))
                            nc.vector.tensor_tensor(out=vt_sb[b][:, ni, :],
                                                    in0=vp, in1=bv_row,
                                                    op=OP.add)

                # --- attention pipeline over units (b, mt) ---
                # scores/exp go in chunk PAIRS with a double-buffered
                # 2-bank tp so ScalarE streams exp back-to-back (the exp
                # of pair p overlaps the score matmuls of pair p+1 and
                # the U matmuls of the previous unit); ScalarE is the
                # critical engine in steady state.
                NP = NCH // 2  # 16 pairs per m-tile
                with tc.tile_pool(name="tpsum", bufs=2, space="PSUM") as tpp, \
                     tc.tile_pool(name="upsum", bufs=4, space="PSUM") as utp:
                    prev_es, prev_xr = None, None
                    for u in range(NU + 1):
                        if u < NU:
                            b, mt = u // NMT, u % NMT
                            ms = slice(mt * MT, (mt + 1) * MT)
                            xr = [xrpool.tile([128, MT], f32, tag=f"xr{h}",
                                              name=f"xr_{u}_{h}")
                                  for h in range(2)]
                            for h in range(2):
                                nc.sync.dma_start(
                                    out=xr[h],
                                    in_=xf_d[b, 128 * h:128 * (h + 1), ms])
                        if u >= 1:
                            pb, pmt = (u - 1) // NMT, (u - 1) % NMT
                            uts = [utp.tile([128, CV], f32, tag="ut",
                                            name=f"ut{i}_{u}")
                                   for i in range(4)]
                        cur_es = {}
                        for p in range(NP):
                            # U matmuls of the previous unit, chunk pair p
                            if u >= 1:
                                ep = prev_es[p]
                                ni = 2 * p
                                st = ni == 0
                                if USE_FP8:
                                    sp = ni == NCH - 2
                                    for i in range(4):
                                        nc.tensor.matmul(
                                            uts[i][:, 0:ucv],
                                            ep[:, 0:2,
                                               128 * i:128 * (i + 1)],
                                            vt_sb[pb][:, ni:ni + 2, 0:ucv],
                                            start=st, stop=sp,
                                            perf_mode=mybir.
                                            MatmulPerfMode.DoubleRow)
                                else:
                                    for jj in range(2):
                                        sp = ni + jj == NCH - 1
                                        for i in range(4):
                                            nc.tensor.matmul(
                                                uts[i],
                                                ep[:, jj,
                                                   128 * i:128 * (i + 1)],
                                                vt_sb[pb][:, ni + jj, :],
                                                start=st and jj == 0,
                                                stop=sp)
                            # scores + exp of the current unit, pair p
                            if u < NU:
                                tp = tpp.tile([128, 2, MT], f32, tag="tp",
                                              name=f"tp_{u}_{p}")
                                for jj in range(2):
                                    ni = 2 * p + jj
                                    jg = 0 if serial_scores else ni % 4
                                    nc.tensor.matmul(
                                        tp[:, jj, :],
                                        k_sb[b][32 * jg:32 * (jg + 1),
                                                ni * 128:(ni + 1) * 128],
                                        q_sb[b][32 * jg:32 * (jg + 1), ms],
                                        start=True, stop=True,
                                        tile_position=(32 * jg, 0))
                                e = epool.tile([128, 2, MT], e_dt, tag="e",
                                               name=f"e_{u}_{p}")
                                if USE_FP8:
                                    if p in dve_pairs:
                                        nc.vector.tensor_scalar(
                                            out=e.bitcast(mybir.dt.uint8),
                                            in0=tp, scalar1=SBIAS,
                                            scalar2=0.0, op0=OP.add,
                                            op1=OP.max)
                                    else:
                                        nc.scalar.activation(e, tp, AF.Exp,
                                                             bias=ebias,
                                                             scale=1.0 / SCL)
                                else:
                                    nc.scalar.activation(e, tp, AF.Exp)
                                cur_es[p] = e

                        # epilogue of the previous unit
                        if u >= 1:
                            pms = pmt * MT
                            for i in range(4):
                                rs = rpool.tile([128, 1], f32, tag="rs")
                                nc.vector.reciprocal(rs, uts[i][:, 256:257])
                                rs2 = rpool.tile([128, 1], f32, tag="rs2")
                                nc.vector.tensor_tensor(out=rs2, in0=rs,
                                                        in1=gamma_b,
                                                        op=OP.mult)
                                t1 = opool.tile([128, C], bf16, tag="t1")
                                nc.vector.tensor_scalar(
                                    out=t1, in0=uts[i][:, 0:C],
                                    scalar1=rs2, scalar2=None, op0=OP.mult)
                                tr = utp.tile([128, 2, 128], bf16, tag="ut",
                                              name=f"tr{i}_{u}")
                                for h in range(2):
                                    nc.tensor.transpose(
                                        tr[:, h, :],
                                        t1[:, 128 * h:128 * (h + 1)], eye_sb)
                                mcs = slice(pms + 128 * i, pms + 128 * (i + 1))
                                for h in range(2):
                                    ot = opool.tile([128, 128], f32,
                                                    tag=f"ot{h}")
                                    nc.vector.tensor_tensor(
                                        out=ot, in0=tr[:, h, :],
                                        in1=prev_xr[h][:, 128 * i:
                                                       128 * (i + 1)],
                                        op=OP.add)
                                    # issue output stores from the (idle)
                                    # gpsimd queue so the sync queue only
                                    # carries the x/xr loads
                                    nc.gpsimd.dma_start(
                                        out=out_d[pb,
                                                  128 * h:128 * (h + 1), mcs],
                                        in_=ot)
                        prev_es, prev_xr = cur_es, xr if u < NU else None

            if repeat == 1:
                body()
            else:
                with tc.For_i(0, repeat, 1):
                    body()

    nc.finalize()
    return nc


_NC_CACHE = {}


def _get_nc():
    if "nc" not in _NC_CACHE:
        _NC_CACHE["nc"] = _build_nc()
    return _NC_CACHE["nc"]


def make_in_maps(inputs, wq, bq, wk, bk, wv, bv, gamma):
    import ml_dtypes
    bf16 = ml_dtypes.bfloat16

    x = np.ascontiguousarray(np.asarray(inputs, np.float32).reshape(B, C, N))
    xb = x.astype(bf16)
    wqT = np.ascontiguousarray(np.asarray(wq, np.float32).T).astype(bf16)
    wkT = np.ascontiguousarray(np.asarray(wk, np.float32).T).astype(bf16)
    wvT_e = np.zeros((C, CV), np.float32)
    wvT_e[:, :C] = np.asarray(wv, np.float32).T
    wvT_e = wvT_e.astype(bf16)
    bv_e = np.zeros((CV,), np.float32)
    bv_e[:C] = np.asarray(bv, np.float32)
    bv_e[C] = 1.0
    bq = np.asarray(bq, np.float32)
    bk = np.asarray(bk, np.float32)
    gamma = np.asarray(gamma, np.float32).reshape(1)
    eye = np.eye(128, dtype=bf16)

    in_maps = []
    for c in range(NCORES):
        sl = slice(c * BPC, (c + 1) * BPC)
        in_maps.append({
            "xb": xb[sl], "xf": x[sl],
            "wqT": wqT, "wkT": wkT, "wvT": wvT_e,
            "bq": bq, "bk": bk, "bv": bv_e, "gamma": gamma,
            "eye": eye,
        })
    return in_maps


def kernel(inputs, wq, bq, wk, bk, wv, bv, gamma):
    from concourse.bass_utils import run_bass_kernel_spmd

    nc = _get_nc()
    in_maps = make_in_maps(inputs, wq, bq, wk, bk, wv, bv, gamma)
    res = run_bass_kernel_spmd(nc, in_maps, core_ids=list(range(NCORES)))
    out = np.concatenate([res.results[c]["out"] for c in range(NCORES)], axis=0)
    return out.reshape(B, C, H, W)


# revision 35
# speedup vs baseline: 1.0236x; 1.0236x over previous
"""CNN attention (nn_CNNAttention_77979426226593) Trainium2 Bass kernel.

Data-parallel over batch: B=16 images -> 8 NeuronCores, 2 images per core.
Each core holds the full (small) conv1x1 weights and computes its local
N x N attention (N = H*W = 4096) independently.

Per image (C=256, N=4096, CQK=32):
  q = wq @ x + bq            [32, N]
  k = wk @ x + bk            [32, N]
  vt = x^T @ [wv^T | 0] + [bv | 1]   [N, 257]  (V transposed, plus a ones
                                     column that rides along as channel 256)
  T[n, m] = k_n . q_m        (scores, N x N, computed n-partitioned)
  E = exp(T)                 (no max-subtraction: logits are small by
                              construction, exp fits fp32/bf16 easily)
  UT[m, c] = sum_n E[n, m] * vt[n, c]   for c in 0..256
       -> UT[:, 0:256] = U^T (attention numerator, m-partitioned)
       -> UT[:, 256]   = d   (softmax denominator) for FREE: the ones
          column of vt adds 1 moving column (~0.4%) instead of a separate
          ones-matmul (which would cost a full third of the U stage,
          since PE matmul time = moving free size, independent of K).
  out[c, m] = (gamma/d[m]) * UT[m, c]^T + x[c, m]

The U matmuls put E (n-partitioned [128,128] chunks) in the STATIONARY
slot and vt in the MOVING slot; PSUM accumulates UT[m, 0:257] over all
32 n-chunks.  The [m, c] -> [c, m] flip at the end is 2 cheap PE
transposes per 128-wide m-chunk (128 cycles each, bf16).

Scores run in bf16 (4-way row-tiled K=32 matmuls -> concurrent PE
quadrants, HW-measured ~190us/exec saving vs serial); U runs in fp8
DoubleRow (E fp8e5 stationary pairs, vt fp8e4 moving, K=256/instr);
accumulation is fp32 in PSUM; softmax normalization is fp32.  The
residual term x is added from a separate fp32 copy, so when gamma == 0
the output equals the input bit-exactly.

Schedule: one "unit" = one (image, 512-wide m-tile).  Unit u runs its
own scores+exp in chunk PAIRS through a double-buffered 2-bank tp tile
(so ScalarE streams exp back-to-back - ScalarE is the pacing engine at
~276us busy/core) while the U matmuls + epilogue of unit u-1 fill the
PE gaps.  PSUM: 4 banks tp (pairs x2) + 4 banks rotating between the 4
UT accumulators and the output transposes.  Output stores issue from
the gpsimd queue so the sync queue only carries input loads (HW: -32us).
"""

import numpy as np

B, C, H, W = 16, 256, 64, 64
N = H * W          # 4096
CQK = 32
NCORES = 8
BPC = B // NCORES  # batches per core

MT = 512           # m tile (attention output columns per score quad)
NMT = N // MT      # 8
NCH = N // 128     # 32 n-chunks (contraction for U)
NQ = NCH // 4      # 8 quads per m-tile
CV = C + 1         # v channels + ones column
NU = BPC * NMT     # pipeline units

# fp8 path: E in fp8e5 (e5m2, exp shifted by E_BIAS so values stay in
# range; the shift cancels exactly in U/d), vt in fp8e4 (e4m3, |v|<~5),
# U matmuls in DoubleRow perf mode (2 fp8 weights per PE cell -> K=256
# per instruction, ~1.5x at FD>=256).  Softmax shift-invariance makes
# the E_BIAS free; accumulation stays fp32 in PSUM.
USE_FP8 = True
# exp(T + E_BIAS): max logit over this input distribution measured 26.4;
# e5m2 max finite is 57344 (ln = 10.96), so -16 keeps exp below ~e^10.4
# with margin, while the weakest softmax columns (col-max ~10) stay in
# e5m2's normal range.  The shift cancels exactly in U/d.
E_BIAS = -16.0
CVPAD = 272        # vt row stride in bytes for DoubleRow (step % 16 == 0)

# ScalarE(exp) is the pacing engine, so a subset of score pairs skips the
# activation entirely: DVE computes the e5m2 BIT PATTERN of exp(T-16)
# directly as uint8 = clamp(T*4*log2(e) + SBIAS, 0, .) — a Schraudolph
# exp whose max rel error (11.6%) equals the e5m2 rounding floor anyway.
# The 4*log2(e) score scale is pre-folded into the q projection (free),
# so the DVE pair costs ONE tensor_scalar(add SBIAS, max 0.0); ScalarE
# undoes the scale with its free activation-scale operand.  The clamp
# keeps the value in [0,120], making HW/sim int-convert edge semantics
# (wrap vs saturate) irrelevant.
SCL = 4 * 1.4426950408889634   # scores arrive as T * SCL
SBIAS = -32.04                 # calibrated for truncating convert
# HW A/B probes showed the DVE offload is a net LOSS (~+13us vs all-Act:
# the DVE read of tp inserts WAR stalls into the score/exp chain and the
# DVE pair costs ~1.6us effective vs ScalarE's 1.1us).  Keep the
# machinery (probes use it) but run everything on ScalarE.
DVE_PAIRS = frozenset()


def _build_nc(repeat=1, dve_pairs=None, u_cols=None, serial_scores=False):
    import contextlib
    import concourse.bacc as bacc
    import concourse.mybir as mybir
    import concourse.tile as tile
    import concourse.bass as bass

    if dve_pairs is None:
        dve_pairs = DVE_PAIRS
    ucv = CV if u_cols is None else u_cols

    f32 = mybir.dt.float32
    bf16 = mybir.dt.bfloat16
    fp8e5 = mybir.dt.float8e5
    fp8e4 = mybir.dt.float8e4
    AF = mybir.ActivationFunctionType
    OP = mybir.AluOpType
    e_dt = fp8e5 if USE_FP8 else bf16
    vt_dt = fp8e4 if USE_FP8 else bf16

    nc = bacc.Bacc("TRN2", target_bir_lowering=False, debug=False,
                   num_devices=NCORES)

    xb_d = nc.dram_tensor("xb", [BPC, C, N], bf16, kind="ExternalInput")
    xf_d = nc.dram_tensor("xf", [BPC, C, N], f32, kind="ExternalInput")
    wqT_d = nc.dram_tensor("wqT", [C, CQK], bf16, kind="ExternalInput")
    wkT_d = nc.dram_tensor("wkT", [C, CQK], bf16, kind="ExternalInput")
    wvT_d = nc.dram_tensor("wvT", [C, CV], bf16, kind="ExternalInput")
    bq_d = nc.dram_tensor("bq", [CQK], f32, kind="ExternalInput")
    bk_d = nc.dram_tensor("bk", [CQK], f32, kind="ExternalInput")
    bv_d = nc.dram_tensor("bv", [CV], f32, kind="ExternalInput")
    gamma_d = nc.dram_tensor("gamma", [1], f32, kind="ExternalInput")
    eye_d = nc.dram_tensor("eye", [128, 128], bf16, kind="ExternalInput")
    out_d = nc.dram_tensor("out", [BPC, C, N], f32, kind="ExternalOutput")

    def bcast_ap(handle, parts, free):
        # DRAM source AP replicated across `parts` partitions (step 0)
        return bass.AP(tensor=handle, offset=0, ap=[[0, parts], [1, free]])

    with tile.TileContext(nc) as tc:
        ctx = contextlib.ExitStack()
        with ctx:
            singles = ctx.enter_context(tc.tile_pool(name="singles", bufs=1))
            xpool = ctx.enter_context(tc.tile_pool(name="xpool", bufs=2))
            qkpool = ctx.enter_context(tc.tile_pool(name="qkpool", bufs=2))
            vtpool = ctx.enter_context(tc.tile_pool(name="vtpool", bufs=2))
            epool = ctx.enter_context(tc.tile_pool(name="epool", bufs=22))
            opool = ctx.enter_context(tc.tile_pool(name="opool", bufs=4))
            xrpool = ctx.enter_context(tc.tile_pool(name="xrpool", bufs=3))
            rpool = ctx.enter_context(tc.tile_pool(name="rpool", bufs=3))

            # --- constants / weights (once) ---
            wqT = singles.tile([C // 2, 2, CQK], bf16, tag="wqT")
            nc.gpsimd.dma_start(out=wqT, in_=wqT_d.ap().rearrange(
                "(t p) o -> p t o", p=128))
            wkT = singles.tile([C // 2, 2, CQK], bf16, tag="wkT")
            nc.gpsimd.dma_start(out=wkT, in_=wkT_d.ap().rearrange(
                "(t p) o -> p t o", p=128))
            wvT = singles.tile([C // 2, 2, CV], bf16, tag="wvT")
            nc.gpsimd.dma_start(out=wvT, in_=wvT_d.ap().rearrange(
                "(t p) o -> p t o", p=128))
            bq_sb = singles.tile([128, 1], f32, tag="bq")
            nc.gpsimd.dma_start(out=bq_sb, in_=bass.AP(
                tensor=bq_d, offset=0, ap=[[0, 4], [1, CQK]]))
            bk_sb = singles.tile([128, 1], f32, tag="bk")
            nc.gpsimd.dma_start(out=bk_sb, in_=bass.AP(
                tensor=bk_d, offset=0, ap=[[0, 4], [1, CQK]]))
            bv_row = singles.tile([128, CV], f32, tag="bvrow")
            nc.gpsimd.dma_start(out=bv_row, in_=bcast_ap(bv_d, 128, CV))
            gamma_b = singles.tile([128, 1], f32, tag="gamma")
            nc.gpsimd.dma_start(out=gamma_b, in_=bcast_ap(gamma_d, 128, 1))
            eye_sb = singles.tile([128, 128], bf16, tag="eye")
            nc.gpsimd.dma_start(out=eye_sb, in_=eye_d.ap())
            ebias = None
            if USE_FP8:
                ebias = singles.tile([128, 1], f32, tag="ebias")
                nc.vector.memset(ebias, E_BIAS)

            def body():
                # --- load x + projections for both images (prologue) ---
                xt = {}
                for b in range(BPC):
                    xt[b] = [xpool.tile([128, N], bf16, tag=f"x{h}",
                                        name=f"xt{h}_{b}") for h in range(2)]
                    for h in range(2):
                        nc.sync.dma_start(
                            out=xt[b][h],
                            in_=xb_d[b, 128 * h:128 * (h + 1), :])

                q_sb, k_sb, vt_sb = {}, {}, {}
                with tc.tile_pool(name="ppsum", bufs=2, space="PSUM") as pp, \
                     tc.tile_pool(name="vpsum", bufs=2, space="PSUM") as vp_:
                    for b in range(BPC):
                        q_sb[b] = qkpool.tile([128, N], bf16, tag="q",
                                              name=f"q_{b}")
                        k_sb[b] = qkpool.tile([128, N], bf16, tag="k",
                                              name=f"k_{b}")
                        vt_sb[b] = vtpool.tile(
                            [128, NCH, CV], vt_dt, tag="vt", name=f"vt_{b}",
                            padded_shape=[128, NCH, CVPAD] if USE_FP8
                            else None)
                        for nt in range(NMT):
                            ns = slice(nt * MT, (nt + 1) * MT)
                            qp = pp.tile([128, MT], f32, tag="qp")
                            for j in range(4):
                                for h in range(2):
                                    nc.tensor.matmul(
                                        qp[32 * j:32 * (j + 1), :],
                                        wqT[:, h, :], xt[b][h][:, ns],
                                        start=(h == 0), stop=(h == 1),
                                        tile_position=(0, 32 * j))
                            nc.vector.tensor_scalar(out=q_sb[b][:, ns], in0=qp,
                                                    scalar1=bq_sb,
                                                    scalar2=SCL if USE_FP8
                                                    else None,
                                                    op0=OP.add,
                                                    op1=OP.mult if USE_FP8
                                                    else ...)
                            kp = pp.tile([128, MT], f32, tag="kp")
                            for j in range(4):
                                for h in range(2):
                                    nc.tensor.matmul(
                                        kp[32 * j:32 * (j + 1), :],
                                        wkT[:, h, :], xt[b][h][:, ns],
                                        start=(h == 0), stop=(h == 1),
                                        tile_position=(0, 32 * j))
                            nc.vector.tensor_scalar(out=k_sb[b][:, ns], in0=kp,
                                                    scalar1=bk_sb, scalar2=None,
                                                    op0=OP.add)
                        for ni in range(NCH):
                            cs = slice(ni * 128, (ni + 1) * 128)
                            vp = vp_.tile([128, CV], f32, tag="vp")
                            for h in range(2):
                                nc.tensor.matmul(vp, xt[b][h][:, cs],
                                                 wvT[:, h, :],
                                                 start=(h == 0), stop=(h == 1))
                            nc.vector.tensor_tensor(out=vt_sb[b][:, ni, :],
                                                    in0=vp, in1=bv_row,
                                                    op=OP.add)

                # --- attention pipeline over units (b, mt) ---
                # scores/exp go in chunk PAIRS with a double-buffered
                # 2-bank tp so ScalarE streams exp back-to-back (the exp
                # of pair p overlaps the score matmuls of pair p+1 and
                # the U matmuls of the previous unit); ScalarE is the
                # critical engine in steady state.
                NP = NCH // 2  # 16 pairs per m-tile
                with tc.tile_pool(name="tpsum", bufs=2, space="PSUM") as tpp, \
                     tc.tile_pool(name="upsum", bufs=4, space="PSUM") as utp:
                    prev_es, prev_xr = None, None
                    for u in range(NU + 1):
                        if u < NU:
                            b, mt = u // NMT, u % NMT
                            ms = slice(mt * MT, (mt + 1) * MT)
                            xr = [xrpool.tile([128, MT], f32, tag=f"xr{h}",
                                              name=f"xr_{u}_{h}")
                                  for h in range(2)]
                            for h in range(2):
                                nc.sync.dma_start(
                                    out=xr[h],
                                    in_=xf_d[b, 128 * h:128 * (h + 1), ms])
                        if u >= 1:
                            pb, pmt = (u - 1) // NMT, (u - 1) % NMT
                            uts = [utp.tile([128, CV], f32, tag="ut",
                                            name=f"ut{i}_{u}")
                                   for i in range(4)]
                        cur_es = {}
                        for p in range(NP):
                            # U matmuls of the previous unit, chunk pair p
                            if u >= 1:
                                ep = prev_es[p]
                                ni = 2 * p
                                st = ni == 0
                                if USE_FP8:
                                    sp = ni == NCH - 2
                                    for i in range(4):
                                        nc.tensor.matmul(
                                            uts[i][:, 0:ucv],
                                            ep[:, 0:2,
                                               128 * i:128 * (i + 1)],
                                            vt_sb[pb][:, ni:ni + 2, 0:ucv],
                                            start=st, stop=sp,
                                            perf_mode=mybir.
                                            MatmulPerfMode.DoubleRow)
                                else:
                                    for jj in range(2):
                                        sp = ni + jj == NCH - 1
                                        for i in range(4):
                                            nc.tensor.matmul(
                                                uts[i],
                                                ep[:, jj,
                                                   128 * i:128 * (i + 1)],
                                                vt_sb[pb][:, ni + jj, :],
                                                start=st and jj == 0,
                                                stop=sp)
                            # scores + exp of the current unit, pair p
                            if u < NU:
                                tp = tpp.tile([128, 2, MT], f32, tag="tp",
                                              name=f"tp_{u}_{p}")
                                for jj in range(2):
                                    ni = 2 * p + jj
                                    jg = 0 if serial_scores else ni % 4
                                    nc.tensor.matmul(
                                        tp[:, jj, :],
                                        k_sb[b][32 * jg:32 * (jg + 1),
                                                ni * 128:(ni + 1) * 128],
                                        q_sb[b][32 * jg:32 * (jg + 1), ms],
                                        start=True, stop=True,
                                        tile_position=(32 * jg, 0))
                                e = epool.tile([128, 2, MT], e_dt, tag="e",
                                               name=f"e_{u}_{p}")
                                if USE_FP8:
                                    if p in dve_pairs:
                                        nc.vector.tensor_scalar(
                                            out=e.bitcast(mybir.dt.uint8),
                                            in0=tp, scalar1=SBIAS,
                                            scalar2=0.0, op0=OP.add,
                                            op1=OP.max)
                                    else:
                                        nc.scalar.activation(e, tp, AF.Exp,
                                                             bias=ebias,
                                                             scale=1.0 / SCL)
                                else:
                                    nc.scalar.activation(e, tp, AF.Exp)
                                cur_es[p] = e

                        # epilogue of the previous unit
                        if u >= 1:
                            pms = pmt * MT
                            for i in range(4):
                                rs = rpool.tile([128, 1], f32, tag="rs")
                                nc.vector.reciprocal(rs, uts[i][:, 256:257])
                                rs2 = rpool.tile([128, 1], f32, tag="rs2")
                                nc.vector.tensor_tensor(out=rs2, in0=rs,
                                                        in1=gamma_b,
                                                        op=OP.mult)
                                t1 = opool.tile([128, C], bf16, tag="t1")
                                nc.vector.tensor_scalar(
                                    out=t1, in0=uts[i][:, 0:C],
                                    scalar1=rs2, scalar2=None, op0=OP.mult)
                                tr = utp.tile([128, 2, 128], bf16, tag="ut",
                                              name=f"tr{i}_{u}")
                                for h in range(2):
                                    nc.tensor.transpose(
                                        tr[:, h, :],
                                        t1[:, 128 * h:128 * (h + 1)], eye_sb)
                                mcs = slice(pms + 128 * i, pms + 128 * (i + 1))
                                for h in range(2):
                                    ot = opool.tile([128, 128], f32,
                                                    tag=f"ot{h}")
                                    nc.vector.tensor_tensor(
                                        out=ot, in0=tr[:, h, :],
                                        in1=prev_xr[h][:, 128 * i:
                                                       128 * (i + 1)],
                                        op=OP.add)
                                    # issue output stores from the (idle)
                                    # gpsimd queue so the sync queue only
                                    # carries the x/xr loads
                                    nc.gpsimd.dma_start(
                                        out=out_d[pb,
                                                  128 * h:128 * (h + 1), mcs],
                                        in_=ot)
                        prev_es, prev_xr = cur_es, xr if u < NU else None

            if repeat == 1:
                body()
            else:
                with tc.For_i(0, repeat, 1):
                    body()

    nc.finalize()
    return nc


_NC_CACHE = {}


def _get_nc():
    if "nc" not in _NC_CACHE:
        _NC_CACHE["nc"] = _build_nc()
    return _NC_CACHE["nc"]


def make_in_maps(inputs, wq, bq, wk, bk, wv, bv, gamma):
    import ml_dtypes
    bf16 = ml_dtypes.bfloat16

    x = np.ascontiguousarray(np.asarray(inputs, np.float32).reshape(B, C, N))
    xb = x.astype(bf16)
    wqT = np.ascontiguousarray(np.asarray(wq, np.float32).T).astype(bf16)
    wkT = np.ascontiguousarray(np.asarray(wk, np.float32).T).astype(bf16)
    wvT_e = np.zeros((C, CV), np.float32)
    wvT_e[:, :C] = np.asarray(wv, np.float32).T
    wvT_e = wvT_e.astype(bf16)
    bv_e = np.zeros((CV,), np.float32)
    bv_e[:C] = np.asarray(bv, np.float32)
    bv_e[C] = 1.0
    bq = np.asarray(bq, np.float32)
    bk = np.asarray(bk, np.float32)
    gamma = np.asarray(gamma, np.float32).reshape(1)
    eye = np.eye(128, dtype=bf16)

    in_maps = []
    for c in range(NCORES):
        sl = slice(c * BPC, (c + 1) * BPC)
        in_maps.append({
            "xb": xb[sl], "xf": x[sl],
            "wqT": wqT, "wkT": wkT, "wvT": wvT_e,
            "bq": bq, "bk": bk, "bv": bv_e, "gamma": gamma,
            "eye": eye,
        })
    return in_maps


def kernel(inputs, wq, bq, wk, bk, wv, bv, gamma):
    from concourse.bass_utils import run_bass_kernel_spmd

    nc = _get_nc()
    in_maps = make_in_maps(inputs, wq, bq, wk, bk, wv, bv, gamma)
    res = run_bass_kernel_spmd(nc, in_maps, core_ids=list(range(NCORES)))
    out = np.concatenate([res.results[c]["out"] for c in range(NCORES)], axis=0)
    return out.reshape(B, C, H, W)
